# revision 51
# baseline (speedup 1.0000x reference)
"""Trainium2 Bass kernel for nn_D2V18AttentionBlock (self-contained).

Sharding: tokens (L=2048) split across 8 cores (T=256 each). One AllGather
exchanges per-core attention-state partial sums. Weights streamed in bf16,
except w1/w2/w3 which run fp8e4(x32 host prescale) DoubleRow matmuls
(2 k-tiles per instruction, 4x PE throughput); descale factors fold into
the silu/copy scales and the final residual add. All weights are host-
retiled so every [128, wide] SBUF tile line is contiguous DRAM.

Activation flow is feature-major [D, T]; the q/k nonlinear middle runs
token-major with q|k fused in one [128, 2D] pass (24 heads). Rmsnorm
scales are applied post-GEMM (per-token scale commutes with the
contraction), so the big GEMMs never wait on a norm chain.

Pipelining: bodies are emitted with a 3-slot staggered scheduler (body i
starts OFFSET chunks after body i-1, round-robin one chunk per tick).
PSUM: 3 suffix tags x 2 bufs + a dedicated norm tag (2) = 8 banks.
Per-body tiles use a per-tag instance modulus m (instances = number of
bodies whose live-range can overlap) to bound SBUF. Engine placement is
HW-measured: PSUM-fed elementwise on DVE, long-Act-wait muls on Pool
(keeps the in-order DVE queue head free), table-based ops on Act grouped
to limit act-table reloads.
"""

import math

import numpy as np
import ml_dtypes

import concourse.bass as bass
import concourse.bacc as bacc
import concourse.mybir as mybir
import concourse.tile as tile
from concourse.bass_utils import run_bass_kernel_spmd

D = 768
H = 12
Dh = 64
LAT = 512
HID = 2048
L = 2048
NCORES = 8
T = L // NCORES          # 256 tokens per core
TB = T // 128            # 2 token-blocks of 128
FD = D // 128            # 6 feature blocks
FL = LAT // 128          # 4 latent blocks
FH = HID // 128          # 16 hidden blocks
STATE_COLS = H // 2 * Dh + H // 2   # 384 P cols + 6 Z cols = 390
NSLOTS = 3                          # bodies in flight (suffix cycle)
OFFSET = 5                          # chunks between body starts

F32 = mybir.dt.float32
F32R = mybir.dt.float32r
BF16 = mybir.dt.bfloat16
F8 = mybir.dt.float8e4
NPF8 = mybir.dt.np(mybir.dt.float8e4)
F8_SCALE = 32.0
NPBF = ml_dtypes.bfloat16
AF = mybir.ActivationFunctionType
OP = mybir.AluOpType


def _f32(x):
    return np.asarray(x, np.float32)


def _bf(x):
    return np.asarray(x, np.float32).astype(NPBF)


def _sigmoid(x):
    return _f32(1.0 / (1.0 + np.exp(-_f32(x), dtype=np.float32)))


def host_prep(inputs):
    """Fold norm weights into matmul weights; build rope/decay tables."""
    x = _f32(inputs["x"]).reshape(L, D)
    w_comp = _f32(inputs["w_comp"]) * _f32(inputs["ln_w"])[:, None]
    w_qkv = _f32(inputs["w_qkv"])
    w_reso = _f32(inputs["w_reso"])
    w_outgate = _f32(inputs["w_outgate"])
    w_proj = _f32(inputs["w_proj"]) * _f32(inputs["memn_w"])[:, None]
    ffn_ln = _f32(inputs["ffn_ln_w"])
    w1 = _f32(inputs["w1"]) * ffn_ln[:, None]
    w2 = _f32(inputs["w2"]) * ffn_ln[:, None]
    w3 = _f32(inputs["w3"])
    temp = float(_f32(inputs["temperature"]).reshape(-1)[0])

    inv_freq = _f32(1.0 / (10000.0 ** (np.arange(0, Dh, 2, dtype=np.float32)
                                       / np.float32(Dh))))
    t_idx = np.arange(L, dtype=np.float32)
    freqs = _f32(np.outer(t_idx, inv_freq))
    emb = np.concatenate([freqs, freqs], -1)
    cos = np.cos(emb, dtype=np.float32)
    sin = np.sin(emb, dtype=np.float32)

    def fold(n_w):
        cos_t = _f32(cos * n_w[None, :])
        # sin table indexed at the SOURCE position: the device computes
        # t2 = x * sin_dev elementwise, then shifts half-blocks. Output
        # position d<32 sources x_{d+32} with weight -sin[d]*n_{d+32}, so
        # sin_dev[d+32] = -sin[d]*n_{d+32}; output d>=32 sources x_{d-32}
        # with weight +sin[d]*n_{d-32}, so sin_dev[d-32] = sin[d]*n_{d-32}.
        sin_dev = sin.copy()
        sin_dev[:, 32:] = _f32(-sin[:, :32] * n_w[None, 32:])
        sin_dev[:, :32] = _f32(sin[:, 32:] * n_w[None, :32])
        return cos_t, sin_dev

    cos_q, sin_q = fold(_f32(inputs["qn_w"]))
    cos_k, sin_k = fold(_f32(inputs["kn_w"]))

    raw_decay = _f32(0.3 + 0.65 * _sigmoid(_f32(inputs["head_decay"])))
    decay_rate = np.clip(raw_decay, 1e-5, 0.999).astype(np.float32)
    dt_h = _f32(1.0 - decay_rate)
    log_decay = np.log(decay_rate, dtype=np.float32)
    cum = np.cumsum(np.broadcast_to(log_decay, (L, H)), axis=0,
                    dtype=np.float32)
    df = np.exp(cum, dtype=np.float32)                       # [L, H]
    wdt = _f32((np.float32(1.0) / _f32(df + np.float32(1e-8))) * dt_h[None, :])

    # indicator: den-reduction lhsT, packed [128, 6*12]
    ind_den = np.zeros((128, 6 * H), np.float32)
    for b in range(6):
        ind_den[0:64, 12 * b + 2 * b] = 1.0
        ind_den[64:128, 12 * b + 2 * b + 1] = 1.0
    # broadcast lhsT [12, 6*128]: block b col p -> 1 if p's head == that row
    ind_bc = np.zeros((H, 6 * 128), np.float32)
    for b in range(6):
        ind_bc[2 * b, 128 * b:128 * b + 64] = 1.0
        ind_bc[2 * b + 1, 128 * b + 64:128 * b + 128] = 1.0

    tri = np.triu(np.ones((128, 128), np.float32))           # [s,t] valid t>=s
    mask0 = np.concatenate([tri, np.ones((128, 128), np.float32)], 1)
    mask1 = np.concatenate([np.zeros((128, 128), np.float32), tri], 1)

    # Retile the column-sliced weights so every [128, w] SBUF tile is one
    # contiguous DRAM block (collapses DMA descriptor count -> cheap issue).
    def quarters(wm):                       # [768, 2048] -> [4*768, 512]
        return np.ascontiguousarray(
            wm.reshape(D, 4, 512).transpose(1, 0, 2).reshape(4 * D, 512))

    def colmajor(wm, wcols=128):            # [R, C] -> [(C/w)*R, w]
        nb = wm.shape[1] // wcols
        return np.ascontiguousarray(
            wm.reshape(wm.shape[0], nb, wcols).transpose(1, 0, 2)
            .reshape(nb * wm.shape[0], wcols))

    def hstack_k(wm):
        """[(K*128), W] -> [128, K*W]: k-blocks side by side, each SBUF
        partition line contiguous in DRAM."""
        K = wm.shape[0] // 128
        return np.ascontiguousarray(
            wm.reshape(K, 128, wm.shape[1]).transpose(1, 0, 2)
            .reshape(128, K * wm.shape[1]))

    w3_t = np.concatenate([w3[:, 0:256], w3[:, 256:512], w3[:, 512:768]],
                          axis=0)                             # [3*HID, 256]
    # w3 tiles: (cp, sub) rows HID*cp+1024*sub..+1024 -> [128, 8*256]
    w3_r = np.concatenate(
        [hstack_k(w3_t[HID * cp + 1024 * sub:HID * cp + 1024 * (sub + 1), :])
         for cp in range(3) for sub in range(2)], axis=0)     # [6*128, 2048]
    segs_a = [(0, 512), (768, 512), (1536, 512)]
    segs_b = [(512, 256), (1280, 256), (2048, 256)]
    w_qkv_a = np.concatenate(
        [hstack_k(w_qkv[:, c:c + w]) for c, w in segs_a], axis=0)
    w_qkv_b = np.concatenate(
        [hstack_k(w_qkv[:, c:c + w]) for c, w in segs_b], axis=0)

    shared = dict(
        w_comp=_bf(hstack_k(w_comp)), w_qkv_a=_bf(w_qkv_a),
        w_qkv_b=_bf(w_qkv_b),
        w_reso=_bf(w_reso),
        w_outgate=_bf(np.concatenate(
            [hstack_k(colmajor(w_outgate)[LAT * 2 * mj:LAT * 2 * (mj + 1), :])
             for mj in range(3)], axis=0)),
        w_proj=_bf(np.concatenate(
            [hstack_k(colmajor(w_proj)[D * 2 * mj:D * 2 * (mj + 1), :])
             for mj in range(3)], axis=0)),
        w1=(np.concatenate(
            [hstack_k(quarters(w1)[D * h:D * (h + 1), :]) for h in range(4)],
            axis=0) * np.float32(F8_SCALE)).astype(NPF8),
        w2=(np.concatenate(
            [hstack_k(quarters(w2)[D * h:D * (h + 1), :]) for h in range(4)],
            axis=0) * np.float32(F8_SCALE)).astype(NPF8),
        w3=(w3_r * np.float32(F8_SCALE)).astype(NPF8),
        ind_bc=ind_bc,
        cpack=np.concatenate([
            ind_den,                                  # 0:72
            mask0,                                    # 72:328
            mask1,                                    # 328:584
        ], axis=1).astype(np.float32),
        cbf=np.concatenate([
            np.ones((128, 1), np.float32),            # 0:1 ones_col
            np.eye(128, dtype=np.float32),            # 1:129 ident
            np.zeros((128, D), np.float32),           # 129:897 zeros
            ind_den,                                  # 897:969 ind_den bf16
            mask0,                                    # 969:1225 tri|ones bf16
            mask1,                                    # 1225:1481 zero|tri bf16
        ], axis=1).astype(NPBF),
        ones_row=np.ones((1, 128), NPBF),
        cfine=np.concatenate([
            np.full((128, 1), temp * 0.5, np.float32),  # 0:1 temp/2
            np.tile(np.concatenate([
                np.array([[1e-6, 1.0 / D, 1.0 / Dh, math.pi, math.pi / 2]],
                         np.float32),
                np.array([0x5F3759DF], np.uint32).view(np.float32)[None, :],
                np.array([[0.5, 0.0]], np.float32)], axis=1), (128, 1)),  # 1:9
            np.zeros((128, T), np.float32),           # 9:265
        ], axis=1).astype(np.float32),
    )
    per_core = []
    for c in range(NCORES):
        t0 = c * T
        mask8 = np.zeros((128, NCORES), np.float32)
        mask8[:, :c] = 1.0
        per_core.append(dict(
            xT=_bf(np.ascontiguousarray(
                x[t0:t0 + T].T.reshape(FD, 128, T).transpose(1, 0, 2)
                .reshape(128, FD * T))),
            tabs=_bf(np.ascontiguousarray(np.concatenate(
                [cos_q[t0:t0 + T], cos_k[t0:t0 + T], sin_q[t0:t0 + T],
                 sin_k[t0:t0 + T]], axis=1))),
            wdt_tm=np.ascontiguousarray(wdt[t0:t0 + T]),
            df_fm=np.ascontiguousarray(df[t0:t0 + T].T),
            mask8=mask8,
        ))
    return shared, per_core


def build_nc(debug=False, loop_n=None, fake_ag_flat=False, unroll=1,
             flat_bodies=1):
    """Build the SPMD Bass program (same for all cores).

    loop_n: timing mode - repeat the body loop_n times via tc.For_i and
    replace the AllGather with equivalent-volume DMA copies (collectives
    cannot sit inside control flow). Numerics are garbage in this mode;
    only the schedule/timing is meaningful.
    """
    import contextlib
    nc = bacc.Bacc("TRN2", target_bir_lowering=False, debug=False,
                   num_devices=NCORES)
    dram = {}

    def din(name, shape, dt=F32):
        dram[name] = nc.dram_tensor(name, list(shape), dt,
                                    kind="ExternalInput").ap()
        return dram[name]

    xT_d = din("xT", (128, FD * T), BF16)
    w_comp_d = din("w_comp", (128, FD * LAT), BF16)
    w_qkv_a_d = din("w_qkv_a", (3 * 128, FL * 512), BF16)
    w_qkv_b_d = din("w_qkv_b", (3 * 128, FL * 256), BF16)
    w_reso_d = din("w_reso", (LAT, 4 * H), BF16)
    w_outgate_d = din("w_outgate", (3 * 128, 2 * 512), BF16)
    w_proj_d = din("w_proj", (3 * 128, 2 * 768), BF16)
    w1_d = din("w1", (4 * 128, FD * 512), F8)
    w2_d = din("w2", (4 * 128, FD * 512), F8)
    w3_d = din("w3", (6 * 128, 8 * 256), F8)
    tabs_d = din("tabs", (T, 4 * Dh), BF16)
    wdt_d = din("wdt_tm", (T, H))
    df_d = din("df_fm", (H, T))
    mask8_d = din("mask8", (128, NCORES))
    cpack_d = din("cpack", (128, 584))
    cbf_d = din("cbf", (128, 129 + D + 72 + 512), BF16)
    ones_row_d = din("ones_row", (1, 128), BF16)
    cfine_d = din("cfine", (128, 9 + T))
    ind_bc_d = din("ind_bc", (H, 6 * 128))

    out_d = nc.dram_tensor("out", [D, T], F32, kind="ExternalOutput").ap()

    dd = locals()
    with tile.TileContext(nc) as tc:
        with nc.allow_low_precision(reason="bf16 compute is intentional"):
            with contextlib.ExitStack() as stack:
                E = _Emitter(nc, tc, dd, stack)
                if loop_n is None:
                    if flat_bodies == 1:
                        E.run_seq(fake_ag=fake_ag_flat, suffix="_u0")
                    else:
                        E.run_staggered(flat_bodies, fake_ag=True)
                elif unroll == 1:
                    with tc.For_i(0, loop_n, 1):
                        E.run_seq(fake_ag=True, suffix="_u0")
                else:
                    assert loop_n % unroll == 0 and unroll % NSLOTS == 0
                    with tc.For_i(0, loop_n // unroll, 1):
                        E.run_staggered(unroll, fake_ag=True)
    nc.compile()
    return nc


class _Emitter:
    """Holds pools/constants created once; body() emits one iteration."""

    def __init__(self, nc, tc, dd, stack):
        self.nc = nc
        self.tc = tc
        self.dd = dd
        v, s, te, sync = nc.vector, nc.scalar, nc.tensor, nc.sync
        self.v, self.s, self.te, self.sync = v, s, te, sync

        def pool(name, space="SBUF"):
            return stack.enter_context(
                tc.tile_pool(name=name, bufs=1, space=space))

        self.const = pool("const")
        self.arena = pool("arena")
        self.psp = pool("psp", space="PSUM")
        self.dram_p = pool("dram", space="DRAM")

        # ---- constants: loaded once, shared by all bodies ----
        const = self.const
        cpack = const.tile([128, 584], F32, name="cpack", tag="cpack")
        sync.dma_start(out=cpack[:].bitcast(F32R),
                       in_=dd["cpack_d"].bitcast(F32R))
        self.ind_den = cpack[:, 0:72]
        self.mask0 = cpack[:, 72:328]
        self.mask1 = cpack[:, 328:584]
        self.mask01 = cpack[:, 72:584]
        cbf = const.tile([128, 129 + D + 72 + 512], BF16, name="cbf",
                         tag="cbf")
        sync.dma_start(out=cbf[:], in_=dd["cbf_d"])
        self.ones_col = cbf[:, 0:1]
        self.ident = cbf[:, 1:129]
        self.zeros_bf = cbf[:, 129:129 + D]
        self.ind_den_bf = cbf[:, 129 + D:129 + D + 72]
        self.mask0_bf = cbf[:, 969:969 + 256]
        self.mask1_bf = cbf[:, 969 + 256:969 + 512]
        self.ones_row = const.tile([1, 128], BF16, name="ones_row",
                                   tag="ones_row")
        sync.dma_start(out=self.ones_row[:], in_=dd["ones_row_d"])
        cfine = const.tile([128, 9 + T], F32, name="cfine", tag="cfine")
        sync.dma_start(out=cfine[:], in_=dd["cfine_d"])
        self.temp_half = cfine[:, 0:1]
        self.cconst = cfine[:, 1:9]
        self.zeros_t = cfine[:, 9:9 + T]
        self.ind_bc = const.tile([H, 6 * 128], F32, name="ind_bc",
                                 tag="ind_bc")
        sync.dma_start(out=self.ind_bc[:].bitcast(F32R),
                       in_=dd["ind_bc_d"].bitcast(F32R))
        self.df_fm = const.tile([H, T], F32, name="df_fm", tag="df_fm")
        sync.dma_start(out=self.df_fm[:], in_=dd["df_d"])
        self.mask8 = const.tile([128, NCORES], F32, name="mask8", tag="mask8")
        sync.dma_start(out=self.mask8[:], in_=dd["mask8_d"])
        self.tabs, self.wdt = [], []
        for tb in range(TB):
            t = const.tile([128, 4 * Dh], BF16, name=f"tabs{tb}",
                           tag=f"tabs{tb}")
            sync.dma_start(out=t[:],
                           in_=dd["tabs_d"][128 * tb:128 * (tb + 1), :])
            self.tabs.append(t)
            w = const.tile([128, H], F32, name=f"wdt{tb}", tag=f"wdt{tb}")
            sync.dma_start(out=w[:],
                           in_=dd["wdt_d"][128 * tb:128 * (tb + 1), :])
            self.wdt.append(w)
        self.suffix = ""
        self.nrm_id = 0

    def run_seq(self, fake_ag, suffix):
        for _ in self.body_gen(fake_ag, 0):
            pass

    def run_staggered(self, n_bodies, fake_ag):
        """Emit n_bodies with NSLOTS-deep software pipelining: body i starts
        OFFSET chunks after body i-1; live bodies advance round-robin one
        chunk per tick, so up to NSLOTS bodies interleave."""
        gens = []
        started = 0
        tick = 0
        while started < n_bodies or gens:
            if started < n_bodies and tick >= started * OFFSET:
                gens.append(self.body_gen(fake_ag, started))
                started += 1
            for g in list(gens):
                try:
                    next(g)
                except StopIteration:
                    gens.remove(g)
            tick += 1

    # ---- tile helpers ----
    def at(self, shape, tag, bufs, dt=F32):
        return self.arena.tile(list(shape), dt, name=tag, tag=tag, bufs=bufs)

    def lt(self, shape, tag, bufs, dt=F32, m=None):
        mod = NSLOTS if m is None else m
        tag = tag + (f"_u{self.body_idx % mod}" if mod > 1 else "")
        return self.arena.tile(list(shape), dt, name=tag, tag=tag, bufs=bufs)

    def pt(self, tag, shape=(128, 512), dt=F32):
        tag = "ps" + self.suffix
        return self.psp.tile(list(shape), dt, name=tag, tag=tag, bufs=2)

    def ptn(self):
        return self.psp.tile([128, 512], F32, name="psn", tag="psn", bufs=2)

    def rmsnorm_fm(self, src_tiles, n, out_tiles, sq_bf, sq_tag, p_tag,
                   cid, sq_pool=False):
        """src feature-major -> out bf16 normalized (Newton rsqrt).

        sq_bf: src tiles already bf16 (sq + out muls run 2x)."""
        v, s, te = self.v, self.s, self.te
        p_ssq = self.ptn()
        for i, xt in enumerate(src_tiles):
            sq = self.at((128, T), sq_tag, 2, BF16)
            (self.nc.gpsimd if sq_pool else v).tensor_mul(sq[:], xt[:], xt[:])
            te.matmul(p_ssq[0:1, 0:T], self.ones_col[:], sq[:],
                      start=(i == 0), stop=(i == len(src_tiles) - 1))
        m = self.at((1, T), "nrms", 14)
        s0 = self.at((1, T), "nrms", 14)
        u = self.at((1, T), "nrms", 14)
        r = self.at((1, T), "nrms", 14)
        m, s0, u, r = m[0:1, :], s0[0:1, :], u[0:1, :], r[0:1, :]
        U32 = mybir.dt.uint32
        v.tensor_scalar(m, p_ssq[0:1, 0:T], (1.0 / n), 1e-6,
                        OP.mult, op1=OP.add)
        s0u = s0.bitcast(U32)
        v.tensor_scalar(s0u, m.bitcast(U32), 1, None,
                        OP.logical_shift_right)
        v.tensor_tensor(s0u,
                        self.cconst[0:1, 5:6].bitcast(U32).broadcast_to(
                            (1, T)), s0u, OP.subtract)
        cur = s0
        for it in range(1):
            v.tensor_mul(u, cur, cur)
            v.scalar_tensor_tensor(u, u, -0.5, m, OP.mult, OP.mult)
            v.scalar_tensor_tensor(r, u, 1.5, cur, OP.add, OP.mult)
            cur = r
        rb = self.at((1, T), f"rbf{cid}", 3, BF16)
        v.tensor_copy(rb[0:1, :], r)
        p_bc = self.ptn()
        te.matmul(p_bc[:, 0:T], self.ones_row[:], rb[0:1, :],
                  start=True, stop=True)
        rbc = self.at((128, T), f"rbc{cid}", 2, BF16)
        s.copy(rbc[:], p_bc[:, 0:T])
        if out_tiles is not None:
            for i, xt in enumerate(src_tiles):
                v.tensor_mul(out_tiles[i][:], xt[:], rbc[:])
        return rbc

    def body_gen(self, fake_ag, body_idx=0):
        nc, dd = self.nc, self.dd
        v, s, te, sync = self.v, self.s, self.te, self.sync
        suffix = f"_u{body_idx % NSLOTS}"
        self.suffix = suffix
        self.body_idx = body_idx

        def Y():
            self.suffix = suffix
            self.body_idx = body_idx
        at, lt, pt = self.at, self.lt, self.pt
        ones_col, ident, zeros_bf = self.ones_col, self.ident, self.zeros_bf
        ones_row, temp_half = self.ones_row, self.temp_half
        cconst = self.cconst
        zeros_t, ind_den, mask0, mask1 = (self.zeros_t, self.ind_den,
                                          self.mask0, self.mask1)
        mask01 = self.mask01
        mask0_bf, mask1_bf = self.mask0_bf, self.mask1_bf
        ind_bc, df_fm, mask8 = self.ind_bc, self.df_fm, self.mask8
        tabs, wdt = self.tabs, self.wdt
        cos2 = [t[:, 0:2 * Dh] for t in tabs]
        sin2 = [t[:, 2 * Dh:4 * Dh] for t in tabs]

        FMN = lambda: at((128, T), "fmn", 9, BF16)

        xwide = lt((128, FD * T), "xld", 1, BF16)
        sync.dma_start(out=xwide[:], in_=dd["xT_d"][:])
        x_fm = [xwide[:, T * i:T * (i + 1)] for i in range(FD)]

        # ---- S1: x rmsnorm scale (rbc only; folded post-GEMM in S2) ----
        rbc1 = self.rmsnorm_fm(x_fm, D, None, sq_bf=True, sq_tag="sq1",
                               p_tag="pe1", cid=0)

        yield; Y()

        # ---- S2: latent (bf16, silu fused on Act engine) ----
        lat_fm = [lt((128, T), f"lat{m}", 1, BF16) for m in range(FL)]
        w2w = at((128, FD * LAT), "wst2", 2, BF16)
        sync.dma_start(out=w2w[:], in_=dd["w_comp_d"][:])
        for mi in range(FL):
            p = pt("pe1")
            for k in range(FD):
                te.matmul(p[:, 0:T],
                          w2w[:, LAT * k + 128 * mi:LAT * k + 128 * (mi + 1)],
                          x_fm[k], start=(k == 0), stop=(k == FD - 1))
            pm = at((128, T), "a2m", 3)
            v.tensor_mul(pm[:], p[:, 0:T], rbc1[:])
            s.activation(lat_fm[mi][:], pm[:], AF.Silu)

        yield; Y()

        # ---- S3: qkv token-major (bf16) ----
        qk_tm = [lt((128, 2 * D), "qk_tm", 2, BF16, m=1)
                 for tb in range(TB)]
        v_tm = [lt((128, D), f"v_tm{tb}", 1, BF16) for tb in range(TB)]
        segs = [(0, 512, "w_qkv_a_d", 0), (512, 256, "w_qkv_b_d", 0),
                (768, 512, "w_qkv_a_d", 1), (1280, 256, "w_qkv_b_d", 1),
                (1536, 512, "w_qkv_a_d", 2), (2048, 256, "w_qkv_b_d", 2)]

        def qkv_dst(col, w):
            if col < 2 * D:
                return ("qk", col)
            return ("v", col - 2 * D)

        for si, (col, w, tn, bi) in enumerate(segs):
            wt = at((128, FL * w), "wsta" if w == 512 else "wstb", 2, BF16)
            sync.dma_start(out=wt[:],
                           in_=dd[tn][128 * bi:128 * (bi + 1), :])
            kind, lc = qkv_dst(col, w)
            for tb in range(TB):
                p = pt("pe1")
                for k in range(FL):
                    te.matmul(p[:, :w],
                              lat_fm[k][:, 128 * tb:128 * (tb + 1)],
                              wt[:, w * k:w * (k + 1)],
                              start=(k == 0), stop=(k == FL - 1))
                dst = {"qk": qk_tm, "v": v_tm}[kind][tb]
                s.copy(dst[:, lc:lc + w], p[:, :w])
            if si == 2:
                yield; Y()

        yield; Y()

        # ---- S4: gate scalars (tanh forms; stays in the silu/sin table) ----
        c_kv = [lt((128, H), f"c_kv{tb}", 1, BF16, m=2) for tb in range(TB)]
        w_reso_sb = lt((128, 4 * H * FL), "w_reso", 1, BF16, m=2)
        for k in range(FL):
            sync.dma_start(out=w_reso_sb[:, 48 * k:48 * (k + 1)],
                           in_=dd["w_reso_d"][128 * k:128 * (k + 1), :])
        AG = lambda: at((128, 48), "agate", 8, BF16)
        sg4s, amps = [], []
        for tb in range(TB):
            p = pt("pe2")
            for k in range(FL):
                te.matmul(p[:, 0:4 * H],
                          lat_fm[k][:, 128 * tb:128 * (tb + 1)],
                          w_reso_sb[:, 48 * k:48 * (k + 1)],
                          start=(k == 0), stop=(k == FL - 1))
            # sigmoid(x) = 0.5*tanh(x/2) + 0.5 (tanh lives in the silu table)
            th4 = AG()
            s.activation(th4[:, 0:4 * H], p[:, 0:4 * H], AF.Tanh,
                         scale=cconst[:, 6:7])
            sg4b = AG()
            v.tensor_scalar(sg4b[:, 0:4 * H], th4[:, 0:4 * H], 0.5, 0.5,
                            OP.mult, op1=OP.add)
            sg4s.append(sg4b)
        for tb in range(TB):
            sg3 = sg4s[tb][:, 0:4 * H].rearrange("p (h f) -> p h f", f=4)
            dphi = AG()
            v.tensor_sub(dphi[:, 0:H], sg3[:, :, 1], sg3[:, :, 3])
            cosv = AG()
            s.activation(cosv[:, 0:H], dphi[:, 0:H], AF.Sin,
                         bias=cconst[:, 4:5], scale=cconst[:, 3:4])
            amp = AG()
            v.tensor_mul(amp[:, 0:H], sg3[:, :, 0], sg3[:, :, 2])
            v.tensor_mul(amp[:, 0:H], amp[:, 0:H], cosv[:, 0:H])
            amps.append(amp)
        for tb in range(TB):
            # base = sigmoid(amp*temp); gate = clip(1.2*base - 0.1, .05, .95)
            #      = clip(0.6*tanh(amp*temp/2) + 0.5, .05, .95)
            base = AG()
            s.activation(base[:, 0:H], amps[tb][:, 0:H], AF.Tanh,
                         scale=temp_half[:])
            g = AG()
            v.tensor_scalar(g[:, 0:H], base[:, 0:H], 0.6, 0.5,
                            OP.mult, op1=OP.add)
            v.tensor_scalar(g[:, 0:H], g[:, 0:H], 0.05, 0.95,
                            OP.max, op1=OP.min)
            v.tensor_mul(c_kv[tb][:], g[:, 0:H], wdt[tb][:])

        yield; Y()

        # ---- S5: fused q|k middle (token-major, bf16, 24 heads) ----
        k_kv_tm = [lt((128, D), f"k_kv_tm{tb}", 1, BF16, m=1) for tb in range(TB)]
        H2 = 2 * H
        D2 = 2 * D
        A3B = lambda: at((128, D2), "a3b", 3, BF16)

        def qk_process(tb, cos_t, sin_t):
            src2d = qk_tm[tb][:, 0:D2]
            src4 = src2d.rearrange("p (g h d) -> p g h d", h=H, d=Dh)
            sq = A3B()
            v.tensor_mul(sq[:, 0:D2], src2d, src2d)
            sm = at((128, 6 * 32), "aqk", 4)
            ss, m, s0, u, r = (sm[:, 0:H2], sm[:, 32:32 + H2],
                               sm[:, 64:64 + H2], sm[:, 96:96 + H2],
                               sm[:, 128:128 + H2])
            v.tensor_reduce(ss, sq[:, 0:D2].rearrange(
                "p (h d) -> p h d", d=Dh), mybir.AxisListType.X, OP.add)
            U32 = mybir.dt.uint32
            v.tensor_scalar(m, ss, (1.0 / Dh), 1e-6, OP.mult, op1=OP.add)
            s0u = s0.bitcast(U32)
            v.tensor_scalar(s0u, m.bitcast(U32), 1, None,
                            OP.logical_shift_right)
            v.tensor_tensor(s0u,
                            cconst[:, 5:6].bitcast(U32).broadcast_to(
                                (128, H2)), s0u, OP.subtract)
            cur = s0
            for it in range(1):
                v.tensor_mul(u, cur, cur)
                v.scalar_tensor_tensor(u, u, -0.5, m, OP.mult, OP.mult)
                v.scalar_tensor_tensor(r, u, 1.5, cur, OP.add, OP.mult)
                cur = r
            qn = A3B()
            qn4 = qn[:, 0:D2].rearrange("p (g h d) -> p g h d", h=H, d=Dh)
            v.tensor_tensor(qn4, src4,
                            r.rearrange("p (g h) -> p g h", h=H)
                            .unsqueeze(3).broadcast_to((128, 2, H, Dh)),
                            OP.mult)
            cos4 = (cos_t[:].rearrange("p (g d) -> p g d", d=Dh)
                    .unsqueeze(2).broadcast_to((128, 2, H, Dh)))
            sin4 = (sin_t[:].rearrange("p (g d) -> p g d", d=Dh)
                    .unsqueeze(2).broadcast_to((128, 2, H, Dh)))
            t1 = A3B()
            t14 = t1[:, 0:D2].rearrange("p (g h d) -> p g h d", h=H, d=Dh)
            v.tensor_tensor(t14, qn4, cos4, OP.mult)
            t2 = A3B()
            t24 = t2[:, 0:D2].rearrange("p (g h d) -> p g h d", h=H, d=Dh)
            v.tensor_tensor(t24, qn4, sin4, OP.mult)
            ro = A3B()
            ro4 = ro[:, 0:D2].rearrange("p (g h d) -> p g h d", h=H, d=Dh)
            v.tensor_tensor(ro4[:, :, :, 0:32], t14[:, :, :, 0:32],
                            t24[:, :, :, 32:64], OP.add)
            v.tensor_tensor(ro4[:, :, :, 32:64], t14[:, :, :, 32:64],
                            t24[:, :, :, 0:32], OP.add)
            mn = A3B()
            v.tensor_scalar_min(mn[:, 0:D2], ro[:, 0:D2], 0.0)
            ex = A3B()
            s.activation(ex[:, 0:D2], mn[:, 0:D2], AF.Exp)
            rel = A3B()
            s.activation(rel[:, 0:D2], ro[:, 0:D2], AF.Relu)
            f = A3B()
            v.tensor_tensor(f[:, 0:D2], rel[:, 0:D2], ex[:, 0:D2], OP.add)
            return f

        z_loc2 = [lt((128, 2 * T), f"zl{i2}", 1, BF16, m=1)
                  for i2 in range(FD // 2)]
        z_loc = [z_loc2[i // 2][:, T * (i % 2):T * (i % 2 + 1)]
                 for i in range(FD)]
        q_fm_w = lt((128, FD * T), "qfm", 1, BF16, m=2)
        kkv_w = lt((128, FD * T), "kkv", 1, BF16, m=2)
        q_fm = [q_fm_w[:, T * i:T * (i + 1)] for i in range(FD)]
        k_kv_fm = [kkv_w[:, T * i:T * (i + 1)] for i in range(FD)]

        def to_fm_tb(tm_tile, fm_wide, tb, width=D):
            nb = width // 128
            p = self.psp.tile([128, 1024], BF16, name="pe2t",
                              tag="ps" + self.suffix, bufs=2)
            for i in range(nb):
                te.transpose(p[:, 128 * i:128 * (i + 1)],
                             tm_tile[:, 128 * i:128 * (i + 1)], ident[:])
            dst = fm_wide[:].rearrange("p (i t) -> p i t", t=T)[
                :, :, 128 * tb:128 * (tb + 1)]
            s.copy(dst, p[:, 0:nb * 128].rearrange("p (i c) -> p i c", c=128))

        kz_tm = [None, None]

        for tb in range(TB):
            qkf = qk_process(tb, cos2[tb], sin2[tb])
            to_fm_tb(qkf[:, 0:D], q_fm_w, tb)
            kf = qkf[:, D:D2]
            v.tensor_tensor(
                k_kv_tm[tb][:].rearrange("p (h d) -> p h d", d=Dh),
                kf.rearrange("p (h d) -> p h d", d=Dh),
                c_kv[tb][:].unsqueeze(2).broadcast_to(
                    (128, H, Dh)), OP.mult)
            to_fm_tb(k_kv_tm[tb][:], kkv_w, tb)
            kz = lt((128, D), f"kztm{tb}", 1, BF16, m=1)
            v.tensor_tensor(
                kz[:].rearrange("p (h d) -> p h d", d=Dh),
                kf.rearrange("p (h d) -> p h d", d=Dh),
                wdt[tb][:].unsqueeze(2).broadcast_to(
                    (128, H, Dh)), OP.mult)
            kz_tm[tb] = kz
            yield; Y()

        # ---- S7: local P state + Z scan ----
        state = lt((128, STATE_COLS), "state", 1, BF16, m=1)
        for b in range(6):
            p = pt("pe2")
            for hh in range(2):
                h = 2 * b + hh
                for tb in range(TB):
                    te.matmul(p[64 * hh:64 * hh + Dh, 0:Dh],
                              k_kv_tm[tb][:, Dh * h:Dh * (h + 1)],
                              v_tm[tb][:, Dh * h:Dh * (h + 1)],
                              start=(tb == 0), stop=(tb == TB - 1))
            s.copy(state[:, Dh * b:Dh * (b + 1)], p[:, 0:Dh])
        for i2 in range(FD // 2):
            pz = pt("pe2")
            for j in range(2):
                i = 2 * i2 + j
                for tb in range(TB):
                    te.matmul(pz[:, T * j:T * (j + 1)],
                              kz_tm[tb][:, 128 * i:128 * (i + 1)],
                              (mask0_bf if tb == 0 else mask1_bf)[:],
                              start=(tb == 0), stop=(tb == TB - 1))
            s.copy(z_loc2[i2][:], pz[:, 0:2 * T])
        for i in range(FD):
            v.tensor_copy(state[:, 6 * Dh + i:6 * Dh + i + 1],
                          z_loc[i][:, T - 1:T])

        # ---- S8: AllGather + prefix ----
        s2 = f"_u{body_idx % 2}"
        st_dram = self.dram_p.tile([128, STATE_COLS], BF16,
                                   name="st_dram" + s2,
                                   tag="st_dram" + s2)
        st_all = self.dram_p.tile([128 * NCORES, STATE_COLS], BF16,
                                  name="st_all" + s2,
                                  tag="st_all" + s2,
                                  addr_space="Local" if fake_ag else "Shared")
        sync.dma_start(out=st_dram[:], in_=state[:])
        if fake_ag:
            for j in range(NCORES):
                sync.dma_start(out=st_all[128 * j:128 * (j + 1), :],
                               in_=st_dram[:])
        else:
            nc.gpsimd.collective_compute(
                "AllGather", OP.bypass,
                replica_groups=[list(range(NCORES))],
                ins=[st_dram[:].opt()], outs=[st_all[:].opt()],
            )
        yield; Y()   # ---- head/tail boundary ----
        # masked prefix accumulation on the (otherwise idle) Pool engine,
        # tree-shaped to shorten the dependency chain
        lv1 = []
        gw = []
        for hh in range(2):
            g4 = at((128, 4 * STATE_COLS), "gst", 2, BF16)
            sync.dma_start(out=g4[:].rearrange("p (j f) -> p j f",
                                               f=STATE_COLS),
                           in_=st_all[512 * hh:512 * (hh + 1), :]
                           .rearrange("(j p) f -> p j f", p=128))
            gw.append(g4)
        for jj in range(4):
            ga = gw[jj // 2][:, STATE_COLS * (2 * (jj % 2)):
                             STATE_COLS * (2 * (jj % 2) + 1)]
            gb = gw[jj // 2][:, STATE_COLS * (2 * (jj % 2) + 1):
                             STATE_COLS * (2 * (jj % 2) + 2)]
            t = at((128, STATE_COLS), "gacc", 4, BF16)
            t2 = at((128, STATE_COLS), "gacc", 4, BF16)
            s.activation(t[:], ga, AF.Copy, scale=mask8[:, 2 * jj:2 * jj + 1])
            s.activation(t2[:], gb, AF.Copy,
                         scale=mask8[:, 2 * jj + 1:2 * jj + 2])
            nc.gpsimd.tensor_add(t[:], t[:], t2[:])
            lv1.append(t)
        nc.gpsimd.tensor_add(lv1[0][:], lv1[0][:], lv1[1][:])
        nc.gpsimd.tensor_add(lv1[2][:], lv1[2][:], lv1[3][:])
        pstate = lt((128, STATE_COLS), "pstate", 1, BF16, m=1)
        nc.gpsimd.tensor_add(pstate[:], lv1[0][:], lv1[2][:])
        pb16 = pstate[:, 0:6 * Dh]

        yield; Y()

        # ---- S9: den path ----
        p_den = pt("pe3")
        for i in range(FD):
            qz = at((128, T), "aqz", 2, BF16)
            v.tensor_mul(qz[:], z_loc[i][:], q_fm[i][:])
            indz = at((128, 16), "indz", 3, BF16)
            nc.gpsimd.tensor_tensor(indz[:, 0:12],
                                    self.ind_den_bf[:, 12 * i:12 * (i + 1)],
                                    pstate[:, 6 * Dh + i:6 * Dh + i + 1]
                                    .broadcast_to((128, 12)), OP.mult)
            te.matmul(p_den[0:H, 0:T],
                      self.ind_den_bf[:, 12 * i:12 * (i + 1)],
                      qz[:], start=(i == 0), stop=False)
            te.matmul(p_den[0:H, 0:T], indz[:, 0:12], q_fm[i][:],
                      start=False, stop=(i == FD - 1))
        den = at((16, T), "s9sm", 3)
        rden = at((16, T), "s9sm", 3)
        dsc = at((16, T), "s9sm", 3)
        sfm = at((16, T), "s9sf", 2)
        v.tensor_mul(den[0:H, :], p_den[0:H, 0:T], df_fm[:])
        v.tensor_scalar_max(den[0:H, :], den[0:H, :], 1e-5)
        v.reciprocal_approx_accurate(rden[0:H, :], den[0:H, :], dsc[0:H, :])
        v.tensor_mul(sfm[0:H, :].bitcast(F32R), rden[0:H, :], df_fm[:])
        sbc = []
        for b in range(6):
            p = pt("pe3")
            te.matmul(p[:, 0:T],
                      ind_bc[:, 128 * b:128 * (b + 1)].bitcast(F32R),
                      sfm[0:H, :].bitcast(F32R), start=True, stop=True)
            o = at((128, T), "sbc", 6, BF16)
            s.copy(o[:], p[:, 0:T])
            sbc.append(o)

        yield; Y()

        # ---- S10: attention ----
        out_fm = [at((128, T), "out_fm", 6, BF16) for _ in range(6)]
        for b in range(6):
            am = {}
            for hh in range(2):
                row0 = 64 * hh
                p_a = pt("pe3")
                for sc in range(TB):
                    te.matmul(p_a[:, T * sc:T * (sc + 1)],
                              k_kv_fm[b][row0:row0 + 64,
                                         128 * sc:128 * (sc + 1)],
                              q_fm[b][row0:row0 + 64, :],
                              start=True, stop=True)
                a = at((128, 2 * T), "am", 3, BF16)
                v.tensor_mul(a[:], p_a[:, 0:2 * T], mask01[:])
                am[hh] = a
            p_on = pt("pe3")
            for hh in range(2):
                h = 2 * b + hh
                row0 = 64 * hh
                for sc in range(TB):
                    te.matmul(p_on[64 * hh:64 * hh + Dh, 0:T],
                              v_tm[sc][:, Dh * h:Dh * (h + 1)],
                              am[hh][:, T * sc:T * (sc + 1)],
                              start=(sc == 0), stop=False)
                te.matmul(p_on[64 * hh:64 * hh + Dh, 0:T],
                          pb16[row0:row0 + 64, Dh * b:Dh * (b + 1)],
                          q_fm[b][row0:row0 + 64, :],
                          start=False, stop=True)
            v.tensor_mul(out_fm[b][:], p_on[:, 0:T], sbc[b][:])
            if b == 2:
                yield; Y()
        yield; Y()

        # ---- S11: memnorm scale (rbc only; folded into outgate in S12) ----
        rbc11 = self.rmsnorm_fm(out_fm, D, None, sq_bf=True, sq_tag="sq11",
                                p_tag="pe3", cid=1)

        yield; Y()

        # ---- S12: attn out + residual ----
        x_mid = [lt((128, T), f"x_mid{mm}", 1, m=1) for mm in range(FD)]
        for mj in range(3):
            wp = at((128, 2 * 768), "w12p", 2, BF16)
            sync.dma_start(out=wp[:],
                           in_=dd["w_proj_d"][128 * mj:128 * (mj + 1), :])
            wo = at((128, 2 * 512), "w12o", 2, BF16)
            sync.dma_start(out=wo[:],
                           in_=dd["w_outgate_d"][128 * mj:128 * (mj + 1), :])
            for j in range(2):
                mi = 2 * mj + j
                p_pr = pt("pe4")
                for k in range(FD):
                    te.matmul(p_pr[:, 0:T],
                              wp[:, 768 * j + 128 * k:768 * j + 128 * (k + 1)],
                              out_fm[k][:],
                              start=(k == 0), stop=(k == FD - 1))
                for k in range(FL):
                    te.matmul(p_pr[:, T:2 * T],
                              wo[:, 512 * j + 128 * k:512 * j + 128 * (k + 1)],
                              lat_fm[k][:],
                              start=(k == 0), stop=(k == FL - 1))
                og = at((128, T), "og", 3, BF16)
                s.activation(og[:], p_pr[:, T:2 * T], AF.Silu)
                xm = at((128, T), "a12", 3)
                v.tensor_mul(xm[:], p_pr[:, 0:T], og[:])
                xm2 = at((128, T), "a12", 3)
                v.tensor_mul(xm2[:], xm[:], rbc11[:])
                nc.gpsimd.tensor_add(x_mid[mi][:], xm2[:], x_fm[mi])

        yield; Y()

        # ---- S13: ffn norm (output straight to fp8 for DoubleRow) ----
        xnf8 = lt((128, FD * T), "xnf8", 1, F8, m=2)
        xnf = [xnf8[:, T * i:T * (i + 1)] for i in range(FD)]
        self.rmsnorm_fm(x_mid, D, xnf, sq_bf=False, sq_tag="sq13",
                        p_tag="pe4", cid=2)

        yield; Y()

        # ---- S14: FFN ----
        h_w = lt((128, FH * T), "fmh", 1, F8, m=1)
        h_fm = [h_w[:, T * i:T * (i + 1)] for i in range(FH)]
        h_w3 = h_w[:].rearrange("p (k t) -> p k t", t=T)
        xnf3 = xnf8[:].rearrange("p (k t) -> p k t", t=T)
        DR = mybir.MatmulPerfMode.DoubleRow
        for half in range(4):
            w1t = at((128, FD * 512), "whq", 5, F8)
            sync.dma_start(out=w1t[:],
                           in_=dd["w1_d"][128 * half:128 * (half + 1), :])
            w2t = at((128, FD * 512), "whq", 5, F8)
            sync.dma_start(out=w2t[:],
                           in_=dd["w2_d"][128 * half:128 * (half + 1), :])
            w1t3 = w1t[:].rearrange("p (k w) -> p k w", w=512)
            w2t3 = w2t[:].rearrange("p (k w) -> p k w", w=512)
            for mj in range(4):
                mi = 4 * half + mj
                p1 = pt("pe4")
                for j in range(FD // 2):
                    te.matmul(p1[:, 0:T],
                              w1t3[:, 2 * j:2 * j + 2,
                                   128 * mj:128 * (mj + 1)],
                              xnf3[:, 2 * j:2 * j + 2, :],
                              start=(j == 0), stop=(j == FD // 2 - 1),
                              perf_mode=DR)
                for j in range(FD // 2):
                    te.matmul(p1[:, T:2 * T],
                              w2t3[:, 2 * j:2 * j + 2,
                                   128 * mj:128 * (mj + 1)],
                              xnf3[:, 2 * j:2 * j + 2, :],
                              start=(j == 0), stop=(j == FD // 2 - 1),
                              perf_mode=DR)
                h1 = at((128, T), "h1", 3, BF16)
                s.activation(h1[:], p1[:, 0:T], AF.Silu,
                             scale=1.0 / F8_SCALE)
                p2c = at((128, T), "h1", 3, BF16)
                s.activation(p2c[:], p1[:, T:2 * T], AF.Copy,
                             scale=16.0 / F8_SCALE)
                nc.gpsimd.tensor_mul(h_fm[mi][:], h1[:], p2c[:])
            if half in (1, 3):
                yield; Y()
        for cp in range(3):
            w3ab = []
            for sub in range(2):
                w3t = at((128, 8 * 256), "w3h", 3, F8)
                sync.dma_start(
                    out=w3t[:],
                    in_=dd["w3_d"][128 * (2 * cp + sub):
                                   128 * (2 * cp + sub + 1), :])
                w3ab.append(w3t)
            for mj in range(2):
                mi = 2 * cp + mj
                p = pt("pe4")
                for sub in range(2):
                    w3t3 = w3ab[sub][:].rearrange("p (k w) -> p k w", w=256)
                    for jj in range(4):
                        te.matmul(p[:, 0:T],
                                  w3t3[:, 2 * jj:2 * jj + 2,
                                       128 * mj:128 * (mj + 1)],
                                  h_w3[:, 8 * sub + 2 * jj:
                                       8 * sub + 2 * jj + 2, :],
                                  start=(sub == 0 and jj == 0),
                                  stop=(sub == 1 and jj == 3),
                                  perf_mode=DR)
                fin = at((128, T), "fin", 2)
                v.scalar_tensor_tensor(fin[:], p[:, 0:T],
                                       1.0 / (F8_SCALE * 16.0),
                                       x_mid[mi][:], OP.mult, OP.add)
                sync.dma_start(out=dd["out_d"][128 * mi:128 * (mi + 1), :],
                               in_=fin[:])


_CACHE = {}


def kernel(**inputs):
    shared, per_core = host_prep(inputs)
    if "nc" not in _CACHE:
        _CACHE["nc"] = build_nc(debug=False)
    nc = _CACHE["nc"]
    in_maps = []
    for c in range(NCORES):
        m = dict(shared)
        m.update(per_core[c])
        in_maps.append(m)
    res = run_bass_kernel_spmd(nc, in_maps, list(range(NCORES)))
    outs = [res.results[c]["out"] for c in range(NCORES)]
    full = np.concatenate([o.T for o in outs], axis=0)
    return full.reshape(1, L, D).astype(np.float32)


if __name__ == "__main__":
    xs = {k: np.asarray(v) for k, v in np.load(
        "/tmp/ref_inputs.npy", allow_pickle=True).item().items()}
    out = kernel(**xs)
    print("out", out.shape, float(np.abs(out).max()))

